# revision 16
# baseline (speedup 1.0000x reference)
"""AttDistance kernel for Trainium2 (8 NeuronCores, SPMD).

Computes, for query (B, Lq, D) and y (B, Ly, D), B=4, Lq=Ly=512, D=128:
    att[b, lq, ly] = -mean_d |query[b, lq, d] - y[b, ly, d]|     (B, Lq, Ly)
    sim[b, 0, lq]  = max_ly att[b, lq, ly]                       (B, 1, Lq)

Sharding: 8 cores = 4 batches x 2 halves of Lq. Each core handles a
(256 lq x 512 ly) block independently; no cross-core communication.

Per-core dataflow (tiles laid out [D=128 partitions, L free], fp16):
  per lq:  |yT - qT[:, lq]| elementwise, then a PE matmul against a
  one-hot ones-column weight valued -1/D reduces over partitions (d),
  landing the (negated, scaled) row att[lq, :] directly in PSUM.

  The elementwise abs-diff is produced by two engines in parallel:
   - VectorE: tensor_scalar pairs relu(y-q) / relu(q-y) (the DVE ISA
     has no single-op abs; the two relu halves sum inside the PE
     accumulation, which is linear).
   - ScalarE: activation(Abs, bias=-q) in one pass.
  PE matmuls are col-tiled (tile_position=(0,32*cg)) so four 32-column
  reductions stream concurrently through the PE array.

  sim = reduce_max over the PSUM tile free dim.
"""

import numpy as np

B, L, D = 4, 512, 128
NCORES = 8
HALF = 256  # lq handled per core

VARIANT = "v3"  # "v3" col-tiled PE | "v3s" serial full-width PE
ACT_ROWS = 12  # of each 32 g-rows, how many go to ScalarE (rest DVE)

_PROG = None


def _build_program(variant=None, act_rows=None, reps=1):
    import concourse.bacc as bacc
    import concourse.tile as tile
    from concourse import mybir

    if variant is None:
        variant = VARIANT
    if act_rows is None:
        act_rows = ACT_ROWS

    f16 = mybir.dt.float16
    f32 = mybir.dt.float32
    A = mybir.AluOpType

    nc = bacc.Bacc("TRN2")
    qT = nc.dram_tensor("qT", [D, HALF], f16, kind="ExternalInput").ap()
    yT = nc.dram_tensor("yT", [D, L], f16, kind="ExternalInput").ap()
    att = nc.dram_tensor("att", [HALF, L], f32, kind="ExternalOutput").ap()
    simo = nc.dram_tensor("simo", [HALF, 1], f32, kind="ExternalOutput").ap()

    # which of the 32 g-rows per chunk go to ScalarE: spread evenly
    act_row = [False] * 32
    if act_rows > 0:
        stride = 32 / act_rows
        for k in range(act_rows):
            act_row[min(31, int(k * stride))] = True

    with tile.TileContext(nc) as tc:
        with (
            tc.tile_pool(name="singles", bufs=1) as singles,
            tc.tile_pool(name="diffs", bufs=16) as diffs,
            tc.tile_pool(name="small", bufs=2) as small,
            tc.tile_pool(name="attsb", bufs=2) as attsb,
            tc.tile_pool(name="psp", bufs=2, space="PSUM") as psp,
        ):
            yT_sb = singles.tile([D, L], f16)
            nc.sync.dma_start(out=yT_sb, in_=yT)
            qT_sb = singles.tile([D, HALF], f16)
            nc.sync.dma_start(out=qT_sb, in_=qT)
            # negated y (fp16) for the relu(q - y) half
            yTn_sb = singles.tile([D, L], f16)
            nc.vector.tensor_scalar_mul(yTn_sb, yT_sb, -1.0)
            # +q / -q in fp32 (tensor_scalar scalar operands, act bias)
            qTn_sb = singles.tile([D, HALF], f32)
            nc.vector.tensor_scalar_mul(qTn_sb, qT_sb, -1.0)
            qTp_sb = singles.tile([D, HALF], f32)
            nc.vector.tensor_scalar_mul(qTp_sb, qT_sb, 1.0)
            # ScalarE fence ops: absorb the DMA / DVE waits one at a time so
            # no downstream Activation needs >1 cross-engine wait after
            # _strip_same_engine_waits.
            fence_sb = singles.tile([D, 1], f32)
            nc.scalar.copy(fence_sb, yT_sb[:, 0:1])
            nc.scalar.copy(fence_sb, qTn_sb[:, 0:1])
            # weight strip: zeros except column D-1 = -1/D. A [D, W] slice
            # starting at offset s has its only nonzero column at D-1-s.
            wstrip = singles.tile([D, 2 * D - 1], f16)
            nc.vector.memset(wstrip, 0.0)
            nc.vector.memset(wstrip[:, D - 1 : D], -1.0 / D)

            def make_diff(lq, on_act):
                if on_act:
                    t = diffs.tile([D, L], f16, tag="dtile")
                    nc.scalar.activation(
                        t,
                        yT_sb,
                        mybir.ActivationFunctionType.Abs,
                        bias=qTn_sb[:, lq : lq + 1],
                        scale=1.0,
                    )
                    return [t]
                t1 = diffs.tile([D, L], f16, tag="dtile")
                nc.vector.tensor_scalar(
                    t1, yT_sb, qTn_sb[:, lq : lq + 1], 0.0, A.add, A.max
                )
                t2 = diffs.tile([D, L], f16, tag="dtile")
                nc.vector.tensor_scalar(
                    t2, yTn_sb, qTp_sb[:, lq : lq + 1], 0.0, A.add, A.max
                )
                return [t1, t2]

            def one_pass():
                for chunk in range(HALF // D):
                    ps = psp.tile([D, L], f32, tag="ps")
                    if variant == "v3":
                        for g in range(32):
                            quad = [
                                make_diff(chunk * D + 32 * cg + g, act_row[g])
                                for cg in range(4)
                            ]
                            nt = len(quad[0])
                            for ti in range(nt):
                                for cg in range(4):
                                    nc.tensor.matmul(
                                        ps[32 * cg : 32 * cg + 32, :],
                                        wstrip[:, D - 1 - g : D - 1 - g + 32],
                                        quad[cg][ti],
                                        start=(g == 0 and ti == 0),
                                        stop=(g == 31 and ti == nt - 1),
                                        tile_position=(0, 32 * cg),
                                        skip_group_check=True,
                                    )
                    elif variant == "v3s":
                        for j in range(D):
                            lq = chunk * D + j
                            tiles = make_diff(lq, act_row[j % 32])
                            for ti, t in enumerate(tiles):
                                nc.tensor.matmul(
                                    ps,
                                    wstrip[:, D - 1 - j : 2 * D - 1 - j],
                                    t,
                                    start=(j == 0 and ti == 0),
                                    stop=(j == D - 1 and ti == len(tiles) - 1),
                                )
                    else:
                        raise ValueError(variant)

                    sim_sb = small.tile([D, 1], f32, tag="sim_sb")
                    nc.vector.tensor_reduce(
                        sim_sb,
                        ps,
                        axis=mybir.AxisListType.X,
                        op=A.max,
                    )
                    att_sb = attsb.tile([D, L], f32, tag="att_sb")
                    nc.scalar.copy(att_sb, ps)
                    nc.sync.dma_start(
                        out=att[chunk * D : (chunk + 1) * D, :], in_=att_sb
                    )
                    nc.sync.dma_start(
                        out=simo[chunk * D : (chunk + 1) * D, :], in_=sim_sb
                    )

            if reps == 1:
                one_pass()
            else:
                # hardware loop: repeat the whole body `reps` times inside
                # the NEFF so wall-clock timing resolves the body duration
                with tc.For_i(0, reps, 1):
                    one_pass()
    return nc


def _strip_same_engine_waits(nc):
    """Remove redundant same-engine semaphore waits.

    Tile emits, on every DVE/ScalarE op, a wait on that engine's own
    semaphore for the immediately preceding op. These engines execute
    and complete their queues in order, so the waits are no-ops - but
    they consume ISA sync-wait slots and force bacc to split them into
    extra EventSemaphore instructions. The increments are kept (other
    engines depend on them); only waits on the instruction's own
    engine's semaphore are dropped. DMA queues are untouched.
    """
    for blk in nc.m.functions[0].blocks:
        for ins in blk.instructions:
            eng = str(ins.engine).split(".")[-1]
            if eng not in ("DVE", "Activation"):
                continue
            si = ins.sync_info
            if si is None:
                continue
            w = si.on_wait
            if not w:
                continue
            keep = [x for x in w if not x.ant_name.startswith(eng + "_")]
            if len(keep) != len(w):
                si.on_wait = keep
    return nc


def _finalize(nc):
    _strip_same_engine_waits(nc)
    nc.compile()
    return nc


def get_program():
    global _PROG
    if _PROG is None:
        _PROG = _finalize(_build_program())
    return _PROG


def make_in_maps(query, y):
    q = np.asarray(query, dtype=np.float32)
    yv = np.asarray(y, dtype=np.float32)
    in_maps = []
    for core in range(NCORES):
        b, h = divmod(core, 2)
        qT = np.ascontiguousarray(q[b, h * HALF : (h + 1) * HALF, :].T).astype(
            np.float16
        )
        yT = np.ascontiguousarray(yv[b].T).astype(np.float16)
        in_maps.append({"qT": qT, "yT": yT})
    return in_maps


def assemble(results):
    att = np.empty((B, L, L), np.float32)
    sim = np.empty((B, 1, L), np.float32)
    for core in range(NCORES):
        b, h = divmod(core, 2)
        att[b, h * HALF : (h + 1) * HALF, :] = results[core]["att"]
        sim[b, 0, h * HALF : (h + 1) * HALF] = results[core]["simo"][:, 0]
    return att, sim


def run_cores(nc, in_maps):
    """Run the (collective-free) SPMD program once per core.

    The 8-device shard_map path in run_bass_kernel_spmd deadlocks during
    XLA compile under this axon build, so dispatch 8 independent
    single-device executions instead - the shards share no data. The
    first call compiles the NEFF; the remaining devices hit the
    neuron compile cache.
    """
    import jax

    from concourse.bass_utils import run_bass_kernel_spmd

    import sys
    import time

    devices = jax.devices()
    results = []
    for core, in_map in enumerate(in_maps):
        t0 = time.time()
        with jax.default_device(devices[core]):
            r = run_bass_kernel_spmd(nc, [in_map], core_ids=[0])
        print(f"[kernel] core {core} done in {time.time()-t0:.1f}s",
              file=sys.stderr, flush=True)
        results.append(r.results[0])
    return results


def kernel(query, y):
    nc = get_program()
    in_maps = make_in_maps(query, y)
    return assemble(run_cores(nc, in_maps))
